# revision 1
# baseline (speedup 1.0000x reference)
"""AdaIN (segment mean/std + EMA of style stats) distributed over 8 TRN2 NeuronCores.

Strategy (data-parallel over rows):
  - content (1M, 64) and style (250K, 64) rows are sharded across 8 cores
    (padded per-core to a multiple of 128 rows; pad rows carry idx=16 which
    no one-hot column matches, so they contribute nothing).
  - pass 1 (per core): per 128-row block, build a one-hot (128,16) of the
    batch index with iota+compare, then one bf16 matmul per block
    accumulates [sum(x) | sum(x^2) | count] per segment into PSUM (16,129).
  - one 16.5KB AllReduce combines per-core partial sums for content+style.
  - stats math on every core (replicated, tiny): mean/std per segment, EMA
    across batch ids via a precomputed 16x16 lower-triangular matrix matmul,
    then per-segment coefficients a = g_std/c_std, b = g_mean - c_mean*a.
  - pass 2 (per core): per 128-row block, gather a/b rows per data row via
    (PE-transposed one-hot) @ [a|b] matmul into PSUM, then vector FMA
    out = x * a_g + b_g, DMA out.
"""

import os
import sys

import numpy as np

for _p in ("/opt/trn_rl_repo",):
    if _p not in sys.path and os.path.isdir(_p):
        sys.path.insert(0, _p)

from concourse import bacc, bass, bass_utils, masks, mybir, tile

F32 = mybir.dt.float32
BF16 = mybir.dt.bfloat16
I32 = mybir.dt.int32

N_CORES = 8
C = 64
B = 16
ALPHA = 0.1
EPS = 1e-8

# full problem sizes (hardcoded per problem spec)
NC_FULL = 1_000_000
NS_FULL = 250_000

# per-core padded rows (multiples of 128)
RC = 128 * 977  # 125056 ; 8*RC = 1000448 >= 1M
RS = 128 * 245  # 31360  ; 8*RS = 250880 >= 250K

BISECT = "full"  # debug: p1only | nocoll | nopass2 | full

SUP = 64  # blocks (of 128 rows) per super-tile
TG2 = 8   # blocks per pass-2 transpose->evict chunk (16x1024 bf16 = 1 PSUM bank)
GCH = 12  # blocks per pass-2 gather/FMA chunk (3 PSUM banks)


def _ema_lhsT() -> np.ndarray:
    """L[b, j] = weight of style-stats row j in globals used by batch b; return L^T."""
    L = np.zeros((B, B), np.float64)
    for b in range(B):
        L[b, 0] = (1.0 - ALPHA) ** b
        for j in range(1, b + 1):
            L[b, j] = ALPHA * (1.0 - ALPHA) ** (b - j)
    return np.ascontiguousarray(L.T).astype(np.float32)


def _chunks(total: int, step: int):
    t0 = 0
    while t0 < total:
        yield t0, min(step, total - t0)
        t0 += step


def build_nc(rc: int = RC, rs: int = RS, n_cores: int = N_CORES):
    """Build + compile the SPMD Bass graph. rc/rs are per-core row counts (mult of 128)."""
    ntc = rc // 128  # content blocks per core
    nts = rs // 128  # style blocks per core

    nc = bacc.Bacc(
        "TRN2", target_bir_lowering=False, debug=False, num_devices=n_cores
    )
    cf = nc.dram_tensor("cf", [rc, C], F32, kind="ExternalInput")
    ci = nc.dram_tensor("ci", [rc], I32, kind="ExternalInput")
    sf = nc.dram_tensor("sf", [rs, C], F32, kind="ExternalInput")
    si = nc.dram_tensor("si", [rs], I32, kind="ExternalInput")
    el = nc.dram_tensor("el", [B, B], F32, kind="ExternalInput")
    out = nc.dram_tensor("out", [rc, C], BF16, kind="ExternalOutput")

    # blocked views: partition p owns rows [p*nt, (p+1)*nt); block t = column t
    cf_v = cf.ap().rearrange("(p n) d -> p n d", p=128)
    ci_v = ci.ap().rearrange("(p n) -> p n", p=128)
    sf_v = sf.ap().rearrange("(p n) d -> p n d", p=128)
    si_v = si.ap().rearrange("(p n) -> p n", p=128)
    out_v = out.ap().rearrange("(p n) d -> p n d", p=128)

    with tile.TileContext(nc) as tc:
        with (
            tc.tile_pool(name="const", bufs=1) as constp,
            tc.tile_pool(name="dram", bufs=1, space="DRAM") as dramp,
        ):
            # --- constants ---
            # iota over 32 columns per block: cols 0:15 match real batch ids,
            # col 16 matches the pad sentinel (routed to zeroed coef rows),
            # cols 17:31 never match (zero pad for 32-aligned transposed strips)
            iota_rep = constp.tile([128, SUP, 2 * B], mybir.dt.int16)
            nc.gpsimd.iota(
                iota_rep[:], pattern=[[0, SUP], [1, 2 * B]], base=0,
                channel_multiplier=0,
            )
            ident = constp.tile([128, 128], BF16)
            masks.make_identity(nc, ident[:])
            el_sb = constp.tile([B, B], F32)
            nc.sync.dma_start(el_sb[:], el.ap())

            # --- resident index tiles ---
            ci_sb = constp.tile([128, ntc], I32)
            nc.sync.dma_start(ci_sb[:], ci_v)
            si_sb = constp.tile([128, nts], I32)
            nc.sync.dma_start(si_sb[:], si_v)

            # content one-hots (32-wide), built in pass 1, reused by pass 2
            oh32 = constp.tile([128, ntc, 2 * B], BF16)

            def pass1(feats_v, idx_sb, nt_total, ps, p1, p1in, oh_cache):
                n_done = 0
                for t0, nt in _chunks(nt_total, SUP):
                    ft = p1in.tile([128, SUP, C], F32, tag="p1f")
                    nc.sync.dma_start(ft[:, :nt, :], feats_v[:, t0 : t0 + nt, :])
                    rhs = p1.tile([128, SUP, 132], BF16, tag="p1r")
                    if oh_cache is not None:
                        oh = oh_cache[:, t0 : t0 + nt, :]
                    else:
                        oh_t = p1.tile([128, SUP, B], BF16, tag="p1oh")
                        oh = oh_t[:, :nt, :]
                    w = oh.shape[-1]
                    # producers at half-super-tile granularity; cast split
                    # DVE/ACT so neither starves the PE for long
                    for k, (h0, nh) in enumerate(_chunks(nt, SUP // 2)):
                        nc.scalar.activation(
                            rhs[:, h0 : h0 + nh, C : 2 * C],
                            ft[:, h0 : h0 + nh, :],
                            mybir.ActivationFunctionType.Square,
                        )
                        nc.vector.tensor_copy(
                            rhs[:, h0 : h0 + nh, 0:C], ft[:, h0 : h0 + nh, :]
                        )
                        nc.vector.memset(
                            rhs[:, h0 : h0 + nh, 2 * C : 2 * C + 1], 1.0
                        )
                        nc.vector.tensor_tensor(
                            oh[:, h0 : h0 + nh, :],
                            idx_sb[:, t0 + h0 : t0 + h0 + nh]
                            .unsqueeze(2)
                            .broadcast_to((128, nh, w)),
                            iota_rep[:, h0 : h0 + nh, 0:w],
                            mybir.AluOpType.is_equal,
                        )
                    for t in range(nt):
                        nc.tensor.matmul(
                            ps[:, 0 : 2 * C + 1],
                            oh[:, t, 0:B],
                            rhs[:, t, 0 : 2 * C + 1],
                            start=(n_done == 0),
                            stop=(n_done == nt_total - 1),
                        )
                        n_done += 1

            # --- stats math helper (tiny, replicated) ---
            def seg_stats(sums, ssq, cnt, mean_out, std_out):
                rc_ = constp.tile([B, 1], F32, tag="t1")
                nc.vector.reciprocal(rc_[:], cnt)
                nm1 = constp.tile([B, 1], F32, tag="t2")
                nc.vector.tensor_scalar_add(nm1[:], cnt, -1.0)
                rnm1 = constp.tile([B, 1], F32, tag="t3")
                nc.vector.reciprocal(rnm1[:], nm1[:])
                fac = constp.tile([B, 1], F32, tag="t4")
                nc.vector.tensor_tensor(fac[:], cnt, rnm1[:], mybir.AluOpType.mult)
                nc.vector.tensor_scalar_mul(mean_out, sums, rc_[:])
                ex2 = constp.tile([B, C], F32, tag="t5")
                nc.vector.tensor_scalar_mul(ex2[:], ssq, rc_[:])
                m2 = constp.tile([B, C], F32, tag="t6")
                nc.vector.tensor_tensor(m2[:], mean_out, mean_out, mybir.AluOpType.mult)
                var = constp.tile([B, C], F32, tag="t7")
                nc.vector.tensor_sub(var[:], ex2[:], m2[:])
                nc.vector.tensor_scalar_mul(var[:], var[:], fac[:])
                nc.vector.tensor_scalar_max(var[:], var[:], 0.0)
                nc.scalar.sqrt(std_out, var[:])
                nc.vector.tensor_scalar_add(std_out, std_out, EPS)

            def all_reduce(src_sb, dst_sb, tag):
                inb = dramp.tile([B, 2 * C + 1], F32, tag=f"in_{tag}")
                outb = dramp.tile([B, 2 * C + 1], F32, tag=f"out_{tag}")
                nc.sync.dma_start(inb[:], src_sb)
                if BISECT == "nocoll":
                    nc.sync.dma_start(outb[:], inb[:])
                else:
                    nc.gpsimd.collective_compute(
                        "AllReduce",
                        mybir.AluOpType.add,
                        replica_groups=[list(range(n_cores))],
                        ins=[inb.opt()],
                        outs=[outb.opt()],
                    )
                nc.sync.dma_start(dst_sb, outb[:])

            do_rest = BISECT != "p1only"
            gm_t = constp.tile([B, C], F32)
            gs_t = constp.tile([B, C], F32)
            # --- pass 1 + per-input stats necks ---
            with (
                tc.tile_pool(name="p1", bufs=4) as p1,
                tc.tile_pool(name="p1in", bufs=3) as p1in,
                tc.tile_pool(name="ps_stats", bufs=1, space="PSUM") as psp,
            ):
                ps_s = psp.tile([B, 2 * C + 1], F32)
                pass1(sf_v, si_sb, nts, ps_s, p1, p1in, None)
                ps_c = psp.tile([B, 2 * C + 1], F32)
                pass1(cf_v, ci_sb, ntc, ps_c, p1, p1in, oh32)
                stats_cs = constp.tile([B, 2 * (2 * C + 1)], F32)
                nc.scalar.copy(stats_cs[:, 0 : 2 * C + 1], ps_c[:, :])
                nc.scalar.copy(stats_cs[:, 2 * C + 1 :], ps_s[:, :])

            if not do_rest:
                nc.sync.dma_start(out.ap()[0:B, 0:C], stats_cs[:, 0:C])

            if do_rest:
              if True:
                g_cs = constp.tile([B, 2 * (2 * C + 1)], F32)
                inb = dramp.tile([B, 2 * (2 * C + 1)], F32)
                outb = dramp.tile([B, 2 * (2 * C + 1)], F32)
                nc.sync.dma_start(inb[:], stats_cs[:])
                if BISECT == "nocoll":
                    nc.sync.dma_start(outb[:], inb[:])
                else:
                    nc.gpsimd.collective_compute(
                        "AllReduce",
                        mybir.AluOpType.add,
                        replica_groups=[list(range(n_cores))],
                        ins=[inb.opt()],
                        outs=[outb.opt()],
                    )
                nc.sync.dma_start(g_cs[:], outb[:])
                g_c = g_cs[:, 0 : 2 * C + 1]
                g_s = g_cs[:, 2 * C + 1 :]
                s_stats = constp.tile([B, 2 * C], F32)  # [mean_s | std_s]
                seg_stats(
                    g_s[:, 0:C], g_s[:, C : 2 * C], g_s[:, 2 * C : 2 * C + 1],
                    s_stats[:, 0:C], s_stats[:, C : 2 * C],
                )
                with tc.tile_pool(name="ps_ema", bufs=1, space="PSUM") as psge:
                    g_ps = psge.tile([B, 2 * C], F32)
                    nc.tensor.matmul(
                        g_ps[:], el_sb[:], s_stats[:], start=True, stop=True
                    )
                    nc.vector.tensor_copy(gm_t[:], g_ps[:, 0:C])
                    nc.vector.tensor_copy(gs_t[:], g_ps[:, C : 2 * C])

                # content stats, shortened chain: a = g_std / sqrt(var_c)
                rc_ = constp.tile([B, 1], F32, tag="t1")
                nc.vector.reciprocal(rc_[:], g_c[:, 2 * C : 2 * C + 1])
                nm1 = constp.tile([B, 1], F32, tag="t2")
                nc.vector.tensor_scalar_add(nm1[:], g_c[:, 2 * C : 2 * C + 1], -1.0)
                rnm1 = constp.tile([B, 1], F32, tag="t3")
                nc.vector.reciprocal(rnm1[:], nm1[:])
                fac = constp.tile([B, 1], F32, tag="t4")
                nc.vector.tensor_tensor(
                    fac[:], g_c[:, 2 * C : 2 * C + 1], rnm1[:], mybir.AluOpType.mult
                )
                mean_c = constp.tile([B, C], F32)
                nc.vector.tensor_scalar_mul(mean_c[:], g_c[:, 0:C], rc_[:])
                ex2 = constp.tile([B, C], F32, tag="t5")
                nc.vector.tensor_scalar_mul(ex2[:], g_c[:, C : 2 * C], rc_[:])
                m2 = constp.tile([B, C], F32, tag="t6")
                nc.scalar.square(m2[:], mean_c[:])
                var = constp.tile([B, C], F32, tag="t7")
                nc.vector.tensor_sub(var[:], ex2[:], m2[:])
                nc.vector.tensor_scalar_mul(var[:], var[:], fac[:])
                std_c = constp.tile([B, C], F32)
                nc.scalar.sqrt(std_c[:], var[:])
                coef = constp.tile([B, 2 * C], BF16)  # [a | b]
                rstd = constp.tile([B, C], F32)
                nc.vector.reciprocal(rstd[:], std_c[:])
                a_t = constp.tile([B, C], F32)
                nc.vector.tensor_tensor(
                    a_t[:], gs_t[:], rstd[:], mybir.AluOpType.mult
                )
                tmp = constp.tile([B, C], F32)
                nc.vector.tensor_tensor(
                    tmp[:], mean_c[:], a_t[:], mybir.AluOpType.mult
                )
                b_t = constp.tile([B, C], F32)
                nc.vector.tensor_sub(b_t[:], gm_t[:], tmp[:])
                nc.vector.tensor_copy(coef[:, 0:C], a_t[:])
                nc.vector.tensor_copy(coef[:, C : 2 * C], b_t[:])

                # block-diag coef for 4-blocks-per-matmul gathers:
                # rows 32q+j (j<16) hold coef[j] at cols [128q, 128q+128);
                # rows 32q+16.. stay zero so pad-sentinel one-hot col 16
                # gathers zeros.
                coef_bd = constp.tile([128, 4 * 2 * C], BF16)
                nc.gpsimd.memset(coef_bd[:], 0.0)
                for q in range(4):
                    nc.sync.dma_start(
                        coef_bd[32 * q : 32 * q + B, 128 * q : 128 * (q + 1)],
                        coef[:],
                    )

                if BISECT != 'nopass2':
                  # --- pass 2 ---
                  with (
                    tc.tile_pool(name="p2", bufs=3) as p2,
                    tc.tile_pool(name="p2in", bufs=4) as p2in,
                    tc.tile_pool(name="ps_t", bufs=2, space="PSUM") as pst,
                    tc.tile_pool(name="ps_g", bufs=2, space="PSUM") as psg2,
                  ):
                    for t0, nt in _chunks(ntc, SUP):
                        f2 = p2in.tile([128, SUP, C], F32, tag="p2f")
                        nc.sync.dma_start(f2[:, :nt, :], cf_v[:, t0 : t0 + nt, :])
                        ot = p2.tile([128, SUP, C], BF16, tag="p2o")
                        for c0, nb in _chunks(nt, GCH):
                            g_ps2 = psg2.tile([128, GCH, 2 * C], F32, tag="gath")
                            ohT_sb = p2.tile([128, (GCH // 4) * 128], BF16, tag="p2ohT")
                            for g0, ng in _chunks(nb, 4):
                                ohT_ps = pst.tile([128, 128], BF16, tag="ohT")
                                nc.tensor.transpose(
                                    ohT_ps[0 : ng * 2 * B, :],
                                    oh32[:, t0 + c0 + g0 : t0 + c0 + g0 + ng, :],
                                    ident[:],
                                )
                                sb_sl = ohT_sb[:, (g0 // 4) * 128 : (g0 // 4) * 128 + 128]
                                nc.scalar.copy(
                                    sb_sl[0 : ng * 2 * B, :], ohT_ps[0 : ng * 2 * B, :]
                                )
                                nc.tensor.matmul(
                                    g_ps2[:, g0 : g0 + ng, :],
                                    sb_sl[0 : ng * 2 * B, :],
                                    coef_bd[0 : ng * 2 * B, 0 : ng * 2 * C],
                                    start=True,
                                    stop=True,
                                )
                            mt = p2.tile([128, GCH, C], F32, tag="p2m")
                            nc.vector.tensor_tensor(
                                mt[:, :nb, :],
                                f2[:, c0 : c0 + nb, :],
                                g_ps2[:, :nb, 0:C],
                                mybir.AluOpType.mult,
                            )
                            nc.vector.tensor_tensor(
                                ot[:, c0 : c0 + nb, :],
                                mt[:, :nb, :],
                                g_ps2[:, :nb, C : 2 * C],
                                mybir.AluOpType.add,
                            )
                        nc.sync.dma_start(out_v[:, t0 : t0 + nt, :], ot[:, :nt, :])

    nc.compile()
    return nc


_NC_CACHE = {}


def _get_nc(rc=RC, rs=RS, n_cores=N_CORES):
    key = (rc, rs, n_cores)
    if key not in _NC_CACHE:
        _NC_CACHE[key] = build_nc(rc, rs, n_cores)
    return _NC_CACHE[key]


def _pad_rows(a: np.ndarray, total: int, fill) -> np.ndarray:
    pad = total - a.shape[0]
    if pad == 0:
        return np.ascontiguousarray(a)
    pad_shape = (pad,) + a.shape[1:]
    return np.concatenate([a, np.full(pad_shape, fill, a.dtype)], axis=0)


def kernel(
    content_feats: np.ndarray,
    style_feats: np.ndarray,
    content_batch_indices: np.ndarray,
    style_batch_indices: np.ndarray,
    num_batches=B,
) -> np.ndarray:
    n_c = content_feats.shape[0]
    n_s = style_feats.shape[0]
    cf = _pad_rows(np.asarray(content_feats, np.float32), N_CORES * RC, 0.0)
    ci = _pad_rows(np.asarray(content_batch_indices, np.int32), N_CORES * RC, B)
    sf = _pad_rows(np.asarray(style_feats, np.float32), N_CORES * RS, 0.0)
    si = _pad_rows(np.asarray(style_batch_indices, np.int32), N_CORES * RS, B)
    el = _ema_lhsT()

    nc = _get_nc()
    in_maps = [
        {
            "cf": np.ascontiguousarray(cf[k * RC : (k + 1) * RC]),
            "ci": np.ascontiguousarray(ci[k * RC : (k + 1) * RC]),
            "sf": np.ascontiguousarray(sf[k * RS : (k + 1) * RS]),
            "si": np.ascontiguousarray(si[k * RS : (k + 1) * RS]),
            "el": el,
        }
        for k in range(N_CORES)
    ]
    res = bass_utils.run_bass_kernel_spmd(nc, in_maps, core_ids=list(range(N_CORES)))
    out = np.concatenate(
        [np.asarray(res.results[k]["out"]) for k in range(N_CORES)], axis=0
    )
    return np.ascontiguousarray(out[:n_c]).astype(np.float32)

